# revision 1
# baseline (speedup 1.0000x reference)
"""Trainium2 Bass kernel for nn_ClusterEncoder (PointTransformerConv-style
GNN message passing), 8-core SPMD.

Strategy (edges sharded by destination node):
  * Host: sort edges by dst, split nodes into 8 equal contiguous ranges
    (edge counts balance to ~0.3% for this random graph). Within a core,
    greedy-pack destination nodes into "chunks" of <=128 nodes and
    <=CHUNK_E edges; pad each chunk's edge list to CHUNK_E slots.
  * Device, phase 1 (replicated): U = x @ (W_dst@Wa1), VH = x @ [W_src@Wa1 | W_lin]
    into DRAM ([N,64] and [N,192]); row-gatherable layouts.
  * Device, phase 2 (per chunk of 16 x 128-edge tiles):
      - gather VH rows by src (768B/row) and U rows by dst,
      - gd = U[dst] - V[src]  (attn-layer-1 folded through node features),
      - pos MLP: t_p1 = relu(Wp1^T posd^T + bp1), delta = relu(Wp2^T t_p1 + bp2),
      - z1 = Wa1^T delta;  t_a = relu(z1 + gd^T + ba1),
      - logits = relu(Wa2^T t_a + ba2);  e = exp(logits - SHIFT)
        (softmax max-subtraction replaced by a constant shift -- exactly
        equivalent math since the shift cancels in e/sum(e); logits are
        relu-bounded so no overflow),
      - one-hot indicator per tile from local dst index (iota + is_equal),
      - segment-sum via matmul: acc[n, 0:128] += ind^T @ (e*(H[src]+delta))^T,
        acc[n, 128:256] += ind^T @ e^T   (numerator and normalizer together),
      - out = relu(NUM / (s + eps)); indirect-scatter rows to y.
  * No collectives needed: softmax segments are core-local by construction.
"""
import sys
from dataclasses import dataclass
from math import ceil

if "/opt/trn_rl_repo" not in sys.path:
    sys.path.insert(0, "/opt/trn_rl_repo")

import numpy as np

import concourse.bass as bass
import concourse.mybir as mybir
import concourse.tile as tile
from concourse import bacc
from concourse.bass import IndirectOffsetOnAxis
from concourse.bass_utils import run_bass_kernel_spmd
from concourse.masks import make_identity

f32 = mybir.dt.float32
f32r = mybir.dt.float32r
i32 = mybir.dt.int32
AF = mybir.ActivationFunctionType
ALU = mybir.AluOpType


@dataclass
class Cfg:
    N: int = 50000
    C: int = 128
    PH: int = 64
    AH: int = 64
    DIM: int = 2
    M: int = 8            # cores
    T: int = 16           # 128-edge tiles per chunk
    TB: int = 4           # tiles per matmul block (block = 512 edges)
    SHIFT: float = 8.0
    EPS: float = 1e-12
    mm_dt: object = f32r  # matmul compute dtype (f32r: 1 cyc/row at free>=256)

    @property
    def NLOC(self):
        return self.N // self.M

    @property
    def CHUNK_E(self):
        return self.T * 128

    @property
    def OUT_ROWS(self):
        return self.NLOC + 1  # +1 trash row for padded scatter lanes


CFG = Cfg()


# ---------------------------------------------------------------- host pack
def _pack(x, pos, edge_index, cfg):
    """Sort/shard/chunk edges; returns per-core input dicts (minus weights)."""
    src = np.asarray(edge_index[0], np.int64)
    dst = np.asarray(edge_index[1], np.int64)
    order = np.argsort(dst, kind="stable")
    s_s = src[order]
    d_s = dst[order]
    posd = (pos[d_s] - pos[s_s]).astype(np.float32)  # [E, 2]

    NLOC = cfg.NLOC
    bounds = np.searchsorted(d_s, np.arange(cfg.M + 1) * NLOC)

    cores = []
    for c in range(cfg.M):
        lo, hi = bounds[c], bounds[c + 1]
        dloc = d_s[lo:hi] - c * NLOC
        deg = np.bincount(dloc, minlength=NLOC)
        nodes = np.nonzero(deg)[0]
        chunks = []  # (node_list, e0, e1) ; e relative to lo
        cur, cur_e, estart = [], 0, 0
        for n in nodes:
            dn = int(deg[n])
            assert dn <= cfg.CHUNK_E, f"degree {dn} exceeds chunk capacity"
            if len(cur) == 128 or cur_e + dn > cfg.CHUNK_E:
                chunks.append((cur, estart, estart + cur_e))
                estart += cur_e
                cur, cur_e = [], 0
            cur.append(int(n))
            cur_e += dn
        if cur:
            chunks.append((cur, estart, estart + cur_e))
        cores.append((lo, chunks, dloc))

    NCHUNK = max(len(ch) for _, ch, _ in cores) if cores else 1
    NCHUNK = max(NCHUNK, 1)

    in_maps = []
    for c in range(cfg.M):
        lo, chunks, dloc = cores[c]
        srcid = np.zeros((NCHUNK, 128, cfg.T), np.int32)
        dstid = np.zeros((NCHUNK, 128, cfg.T), np.int32)
        dstloc = np.full((NCHUNK, 128, cfg.T), -1.0, np.float32)
        posdT = np.zeros((NCHUNK, cfg.DIM, cfg.CHUNK_E), np.float32)
        outrow = np.full((NCHUNK, 128), cfg.NLOC, np.int32)  # trash row
        for k, (nl, e0, e1) in enumerate(chunks):
            cnt = e1 - e0
            g0, g1 = lo + e0, lo + e1
            nla = np.asarray(nl, np.int64)
            loc = np.searchsorted(nla, dloc[e0:e1]).astype(np.float32)
            j = np.arange(cnt)
            t_idx = j >> 7
            lane = j & 127
            srcid[k, lane, t_idx] = s_s[g0:g1].astype(np.int32)
            dstid[k, lane, t_idx] = d_s[g0:g1].astype(np.int32)
            dstloc[k, lane, t_idx] = loc
            posdT[k, :, :cnt] = posd[g0:g1].T
            outrow[k, : len(nl)] = nla.astype(np.int32)
        in_maps.append(
            dict(srcid=srcid, dstid=dstid, dstloc=dstloc, posdT=posdT,
                 outrow=outrow)
        )
    return in_maps, NCHUNK


# ---------------------------------------------------------------- program
def _build(cfg, nchunk):
    nc = bacc.Bacc(None, target_bir_lowering=False)
    N, C, PH, AH, DIM = cfg.N, cfg.C, cfg.PH, cfg.AH, cfg.DIM
    mdt = cfg.mm_dt

    x_d = nc.declare_dram_parameter("x", [N, C], f32, isOutput=False)
    wnode_d = nc.declare_dram_parameter("Wnode", [C, 2 * AH + C], f32, isOutput=False)
    wp1_d = nc.declare_dram_parameter("Wp1", [DIM, PH], f32, isOutput=False)
    wp2_d = nc.declare_dram_parameter("Wp2", [PH, C], f32, isOutput=False)
    wa1_d = nc.declare_dram_parameter("Wa1p", [C, AH], f32, isOutput=False)
    wa2_d = nc.declare_dram_parameter("Wa2", [AH, C], f32, isOutput=False)
    bias_d = nc.declare_dram_parameter("bias", [128, 5], f32, isOutput=False)
    src_d = nc.declare_dram_parameter("srcid", [nchunk, 128, cfg.T], i32, isOutput=False)
    dst_d = nc.declare_dram_parameter("dstid", [nchunk, 128, cfg.T], i32, isOutput=False)
    dl_d = nc.declare_dram_parameter("dstloc", [nchunk, 128, cfg.T], f32, isOutput=False)
    pd_d = nc.declare_dram_parameter("posdT", [nchunk, DIM, cfg.CHUNK_E], f32, isOutput=False)
    or_d = nc.declare_dram_parameter("outrow", [nchunk, 128], i32, isOutput=False)
    y_d = nc.declare_dram_parameter("y", [cfg.OUT_ROWS, C], f32, isOutput=True)

    U_d = nc.dram_tensor("U", [N, AH], f32)          # x @ (W_dst@Wa1)
    VH_d = nc.dram_tensor("VH", [N, AH + C], f32)    # x @ [W_src@Wa1 | W_lin]

    NB = cfg.T // cfg.TB  # blocks per chunk
    BLK = cfg.TB * 128

    with tile.TileContext(nc) as tc:
        with tc.tile_pool(name="const", bufs=1) as cp:
            wnode_s = cp.tile([C, 2 * AH + C], f32)
            nc.sync.dma_start(out=wnode_s[:], in_=wnode_d[:, :])
            wp1_s = cp.tile([DIM, PH], f32)
            nc.sync.dma_start(out=wp1_s[:], in_=wp1_d[:, :])
            wp2_s = cp.tile([PH, C], f32)
            nc.sync.dma_start(out=wp2_s[:], in_=wp2_d[:, :])
            wa2_s = cp.tile([AH, C], f32)
            nc.sync.dma_start(out=wa2_s[:], in_=wa2_d[:, :])
            bias_s = cp.tile([128, 5], f32)
            nc.sync.dma_start(out=bias_s[:], in_=bias_d[:, :])
            ident_s = cp.tile([128, 128], f32)
            make_identity(nc, ident_s[:])
            iota_i = cp.tile([128, 128], i32)
            nc.gpsimd.iota(iota_i[:], pattern=[[1, 128]], base=0, channel_multiplier=0)
            iota_s = cp.tile([128, 128], f32)
            nc.vector.tensor_copy(iota_s[:], iota_i[:])
            wa1_s = cp.tile([C, AH], f32)
            nc.sync.dma_start(out=wa1_s[:], in_=wa1_d[:, :])

            # fp32r matmul operands must be produced rounded-to-f32r: make
            # rounded copies of the stationary weights once.
            if mdt is f32r:
                wnode_m = cp.tile([C, 2 * AH + C], f32r)
                nc.vector.tensor_copy(wnode_m[:], wnode_s[:])
                wp1_m = cp.tile([DIM, PH], f32r)
                nc.vector.tensor_copy(wp1_m[:], wp1_s[:])
                wp2_m = cp.tile([PH, C], f32r)
                nc.vector.tensor_copy(wp2_m[:], wp2_s[:])
                wa1_m = cp.tile([C, AH], f32r)
                nc.vector.tensor_copy(wa1_m[:], wa1_s[:])
                wa2_m = cp.tile([AH, C], f32r)
                nc.vector.tensor_copy(wa2_m[:], wa2_s[:])
            else:
                wnode_m, wp1_m, wp2_m, wa1_m, wa2_m = wnode_s, wp1_s, wp2_s, wa1_s, wa2_s

            # ---------------- phase 1: node features U / VH ----------------
            with tc.tile_pool(name="p1", bufs=3) as p1, \
                 tc.tile_pool(name="p1ps", bufs=2, space="PSUM") as p1ps:
                nt = ceil(N / 128)
                for t in range(nt):
                    r0 = t * 128
                    rows = min(128, N - r0)
                    xt = p1.tile([128, C], f32, tag="xt")
                    nc.sync.dma_start(out=xt[:rows], in_=x_d[r0:r0 + rows, :])
                    xT_p = p1ps.tile([128, 128], f32, tag="xT")
                    nc.tensor.transpose(xT_p[:, :rows], xt[:rows, :], ident_s[:rows, :rows])
                    xT_s = p1.tile([128, 128], mdt, tag="xTs")
                    nc.vector.tensor_copy(xT_s[:, :rows], xT_p[:, :rows])
                    uvh_p = p1ps.tile([128, 2 * AH + C], f32, tag="uvh")
                    nc.tensor.matmul(uvh_p[:rows, :], lhsT=xT_s[:, :rows],
                                     rhs=wnode_m[:], start=True, stop=True)
                    uvh_s = p1.tile([128, 2 * AH + C], f32, tag="uvhs")
                    nc.scalar.activation(uvh_s[:rows, :], uvh_p[:rows, :], AF.Copy)
                    nc.sync.dma_start(out=U_d[r0:r0 + rows, :], in_=uvh_s[:rows, 0:AH])
                    nc.sync.dma_start(out=VH_d[r0:r0 + rows, :], in_=uvh_s[:rows, AH:])

            # ---------------- phase 2: edges ----------------
            with tc.tile_pool(name="eb", bufs=3) as eb, \
                 tc.tile_pool(name="ebg", bufs=3) as ebg, \
                 tc.tile_pool(name="ps_acc", bufs=2, space="PSUM") as ps_acc, \
                 tc.tile_pool(name="ps_b", bufs=1, space="PSUM") as ps_b, \
                 tc.tile_pool(name="ps_c", bufs=1, space="PSUM") as ps_c, \
                 tc.tile_pool(name="ps_m", bufs=1, space="PSUM") as ps_m, \
                 tc.tile_pool(name="ps_n", bufs=1, space="PSUM") as ps_n, \
                 tc.tile_pool(name="ps_t", bufs=2, space="PSUM") as ps_t:
                for k in range(nchunk):
                    src_s = eb.tile([128, cfg.T], i32, tag="src")
                    nc.sync.dma_start(out=src_s[:], in_=src_d[k, :, :])
                    dst_s = eb.tile([128, cfg.T], i32, tag="dst")
                    nc.sync.dma_start(out=dst_s[:], in_=dst_d[k, :, :])
                    dl_s = eb.tile([128, cfg.T], f32, tag="dl")
                    nc.sync.dma_start(out=dl_s[:], in_=dl_d[k, :, :])
                    pd_s = eb.tile([DIM, cfg.CHUNK_E], f32, tag="pd")
                    nc.sync.dma_start(out=pd_s[:], in_=pd_d[k, :, :])
                    if mdt is f32r:
                        pd_m = eb.tile([DIM, cfg.CHUNK_E], f32r, tag="pdm")
                        nc.vector.tensor_copy(pd_m[:], pd_s[:])
                    else:
                        pd_m = pd_s
                    or_s = eb.tile([128, 1], i32, tag="or")
                    nc.sync.dma_start(out=or_s[:], in_=or_d[k, :, None])

                    acc_p = ps_acc.tile([128, 2 * C], f32, tag="acc")

                    for b in range(NB):
                        esl = slice(b * BLK, (b + 1) * BLK)
                        # gathers for this block, one [128,1]-offset DMA per tile
                        vhgs, ugs = [], []
                        for tt in range(cfg.TB):
                            ti = b * cfg.TB + tt
                            vhg_t = ebg.tile([128, AH + C], f32, tag=f"vhg{tt}")
                            nc.gpsimd.indirect_dma_start(
                                out=vhg_t[:], out_offset=None, in_=VH_d[:],
                                in_offset=IndirectOffsetOnAxis(
                                    ap=src_s[:, ti:ti + 1], axis=0))
                            vhgs.append(vhg_t)
                            ug_t = ebg.tile([128, AH], f32, tag=f"ug{tt}")
                            nc.gpsimd.indirect_dma_start(
                                out=ug_t[:], out_offset=None, in_=U_d[:],
                                in_offset=IndirectOffsetOnAxis(
                                    ap=dst_s[:, ti:ti + 1], axis=0))
                            ugs.append(ug_t)

                        # pos MLP
                        tp1_p = ps_m.tile([PH, BLK], f32, tag="tp1")
                        nc.tensor.matmul(tp1_p[:], lhsT=wp1_m[:],
                                         rhs=pd_m[:, esl], start=True, stop=True)
                        tp1_s = eb.tile([PH, BLK], mdt, tag="tp1s")
                        nc.scalar.activation(tp1_s[:], tp1_p[:], AF.Relu, bias=bias_s[0:PH, 0:1])
                        del_p = ps_b.tile([C, BLK], f32, tag="delp")
                        nc.tensor.matmul(del_p[:], lhsT=wp2_m[:],
                                         rhs=tp1_s[:], start=True, stop=True)
                        del_s = eb.tile([C, BLK], f32, tag="dels")
                        nc.scalar.activation(del_s[:], del_p[:], AF.Relu, bias=bias_s[:, 1:2])
                        if mdt is f32r:
                            del_m = eb.tile([C, BLK], f32r, tag="delm")
                            nc.scalar.activation(del_m[:], del_p[:], AF.Relu, bias=bias_s[:, 1:2])
                        else:
                            del_m = del_s

                        # attn layer 1: z1 = Wa1^T delta ; t_a = relu(z1 + gd^T + ba1)
                        z1_p = ps_n.tile([AH, BLK], f32, tag="z1")
                        nc.tensor.matmul(z1_p[:], lhsT=wa1_m[:],
                                         rhs=del_m[:], start=True, stop=True)
                        tsum_s = eb.tile([AH, BLK], f32, tag="tsum")
                        for tt in range(cfg.TB):
                            gd_s = eb.tile([128, AH], f32, tag="gd")
                            nc.vector.tensor_tensor(gd_s[:], ugs[tt][:], vhgs[tt][:, 0:AH],
                                                    op=ALU.subtract)
                            gdT_p = ps_t.tile([128, 128], f32, tag="tr")
                            nc.tensor.transpose(gdT_p[:AH, :], gd_s[:], ident_s[:])
                            gdT_s = eb.tile([AH, 128], f32, tag="gdT")
                            nc.scalar.activation(gdT_s[:], gdT_p[:AH, :], AF.Copy)
                            csl = slice(tt * 128, (tt + 1) * 128)
                            nc.vector.tensor_tensor(tsum_s[:, csl], z1_p[:, csl],
                                                    gdT_s[:], op=ALU.add)
                        ta_s = eb.tile([AH, BLK], mdt, tag="ta")
                        nc.scalar.activation(ta_s[:], tsum_s[:], AF.Relu, bias=bias_s[0:AH, 2:3])

                        # attn layer 2 + exp
                        al_p = ps_c.tile([C, BLK], f32, tag="al")
                        nc.tensor.matmul(al_p[:], lhsT=wa2_m[:],
                                         rhs=ta_s[:], start=True, stop=True)
                        ar_s = eb.tile([C, BLK], f32, tag="ar")
                        nc.scalar.activation(ar_s[:], al_p[:], AF.Relu, bias=bias_s[:, 3:4])
                        e_s = eb.tile([C, BLK], f32, tag="e")
                        nc.scalar.activation(e_s[:], ar_s[:], AF.Exp, bias=bias_s[:, 4:5])
                        ew2_s = eb.tile([C, BLK], f32, tag="ew2")
                        nc.vector.tensor_tensor(ew2_s[:], e_s[:], del_s[:], op=ALU.mult)
                        del del_s  # f32 copy only feeds ew2

                        # per-tile: transpose, assemble [ew | e]^T, indicator, seg-matmul
                        for tt in range(cfg.TB):
                            ti = b * cfg.TB + tt
                            csl = slice(tt * 128, (tt + 1) * 128)
                            eT_p = ps_t.tile([128, 128], f32, tag="tr")
                            nc.tensor.transpose(eT_p[:], e_s[:, csl], ident_s[:])
                            ew2T_p = ps_t.tile([128, 128], f32, tag="tr")
                            nc.tensor.transpose(ew2T_p[:], ew2_s[:, csl], ident_s[:])
                            ewe_s = eb.tile([128, 2 * C], mdt, tag="ewe")
                            nc.vector.tensor_copy(ewe_s[:, C:], eT_p[:])
                            tmp_s = eb.tile([128, C], f32, tag="tmp")
                            nc.vector.tensor_tensor(tmp_s[:], eT_p[:], vhgs[tt][:, AH:],
                                                    op=ALU.mult)
                            nc.vector.tensor_tensor(ewe_s[:, 0:C], tmp_s[:], ew2T_p[:],
                                                    op=ALU.add)
                            ind_s = eb.tile([128, 128], mdt, tag="ind")
                            nc.vector.tensor_scalar(ind_s[:], iota_s[:], dl_s[:, ti:ti + 1],
                                                    None, op0=ALU.is_equal)
                            nc.tensor.matmul(acc_p[:], lhsT=ind_s[:],
                                             rhs=ewe_s[:],
                                             start=(ti == 0), stop=(ti == cfg.T - 1))

                    # finalize chunk
                    sp_s = eb.tile([128, C], f32, tag="sp")
                    nc.vector.tensor_scalar_add(sp_s[:], acc_p[:, C:], cfg.EPS)
                    rp_s = eb.tile([128, C], f32, tag="rp")
                    nc.vector.reciprocal(rp_s[:], sp_s[:])
                    o_s = eb.tile([128, C], f32, tag="o")
                    nc.vector.tensor_tensor(o_s[:], acc_p[:, 0:C], rp_s[:], op=ALU.mult)
                    o2_s = eb.tile([128, C], f32, tag="o2")
                    nc.scalar.activation(o2_s[:], o_s[:], AF.Relu)
                    nc.gpsimd.indirect_dma_start(
                        out=y_d[:], out_offset=IndirectOffsetOnAxis(ap=or_s[:, :1], axis=0),
                        in_=o2_s[:], in_offset=None)
    nc.finalize()
    return nc


def _build_inputs(inputs, cfg):
    x = np.ascontiguousarray(np.asarray(inputs["x"], np.float32))
    pos = np.ascontiguousarray(np.asarray(inputs["pos"], np.float32))
    W_lin = np.asarray(inputs["W_lin"], np.float32)
    W_src = np.asarray(inputs["W_src"], np.float32)
    W_dst = np.asarray(inputs["W_dst"], np.float32)
    Wp1 = np.asarray(inputs["Wp1"], np.float32)
    bp1 = np.asarray(inputs["bp1"], np.float32)
    Wp2 = np.asarray(inputs["Wp2"], np.float32)
    bp2 = np.asarray(inputs["bp2"], np.float32)
    Wa1 = np.asarray(inputs["Wa1"], np.float32)
    ba1 = np.asarray(inputs["ba1"], np.float32)
    Wa2 = np.asarray(inputs["Wa2"], np.float32)
    ba2 = np.asarray(inputs["ba2"], np.float32)

    Wda = (W_dst @ Wa1).astype(np.float32)   # [C, AH]
    Wsa = (W_src @ Wa1).astype(np.float32)
    wnode = np.concatenate([Wda, Wsa, W_lin], axis=1)  # [C, 2AH + C]
    bias = np.zeros((128, 5), np.float32)
    bias[: cfg.PH, 0] = bp1
    bias[: cfg.C, 1] = bp2
    bias[: cfg.AH, 2] = ba1
    bias[: cfg.C, 3] = ba2
    bias[:, 4] = -cfg.SHIFT

    packs, nchunk = _pack(x, pos, inputs["edge_index"], cfg)
    common = dict(x=x, Wnode=np.ascontiguousarray(wnode),
                  Wp1=np.ascontiguousarray(Wp1), Wp2=np.ascontiguousarray(Wp2),
                  Wa2=np.ascontiguousarray(Wa2), bias=bias)
    common["Wa1p"] = np.ascontiguousarray(Wa1)
    in_maps = [dict(common, **p) for p in packs]
    return in_maps, nchunk


def kernel(**inputs):
    cfg = CFG
    in_maps, nchunk = _build_inputs(inputs, cfg)
    nc = _build(cfg, nchunk)
    res = run_bass_kernel_spmd(nc, in_maps, list(range(cfg.M)))
    y = np.concatenate([res.results[c]["y"][: cfg.NLOC] for c in range(cfg.M)], axis=0)
    return y



# revision 7
# speedup vs baseline: 3.6963x; 3.6963x over previous
"""Trainium2 Bass kernel for nn_ClusterEncoder (PointTransformerConv-style
GNN message passing), 8-core SPMD.

Strategy (edges sharded by destination node; fp16 data plane):
  * Host: sort edges by dst, split nodes into 8 equal contiguous ranges
    (edge counts balance to ~0.3% for this random graph). Within a core,
    greedy-pack destination nodes into "chunks" of <=128 nodes and
    <=CHUNK_E edges; pad each chunk's edge list to CHUNK_E slots.
    Each core receives ONLY its node shard (xT, fp16, transposed) plus
    its edge maps — ~3.5 MB/core instead of a replicated 25.6 MB x.
  * Device, phase 1 (local shard only): U_loc = x_c @ (W_dst@Wa1) and
    VH_loc = x_c @ [W_src@Wa1 | W_lin] for the core's own 6250 nodes.
  * AllGather VH_loc across the 8 cores -> vh_full [N, 192] fp16
    (contiguous node shards concatenate rank-major, so global src ids
    index it directly). U stays local: dst ids are core-local by the
    edge sharding, so the U gather reads the local table.
  * Device, phase 2 (per chunk of 16 x 128-edge tiles):
      - gather VH rows by src (384B/row) and U rows by local dst,
      - pos MLP: t_p1 = relu(Wp1^T posd^T + bp1), delta = relu(Wp2^T t_p1 + bp2),
      - z1 = Wa1^T delta;  t_a = relu(z1 + (U[dst]-V[src])^T + ba1),
      - logits = relu(Wa2^T t_a + ba2);  e = exp(logits - SHIFT)
        (softmax max-subtraction replaced by a constant shift -- exactly
        equivalent math since the shift cancels in e/sum(e); logits are
        relu-bounded so no overflow),
      - one-hot indicator per tile from local dst index (iota + is_equal),
      - segment-sum via matmul: acc[n, 0:128] += ind^T @ (e*(H[src]+delta))^T,
        acc[n, 128:256] += ind^T @ e^T   (numerator and normalizer together),
      - out = relu(NUM / (s + eps)); indirect-scatter fp16 rows to y.
  * Softmax segments are core-local by construction, so the only
    collective is the single VH AllGather.
"""
import sys
from dataclasses import dataclass
from math import ceil

if "/opt/trn_rl_repo" not in sys.path:
    sys.path.insert(0, "/opt/trn_rl_repo")

import numpy as np

import concourse.bass as bass
import concourse.mybir as mybir
import concourse.tile as tile
from concourse import bacc
from concourse.bass import IndirectOffsetOnAxis
from concourse.bass_utils import run_bass_kernel_spmd
from concourse.masks import make_identity

f32 = mybir.dt.float32
f16 = mybir.dt.float16
i32 = mybir.dt.int32
AF = mybir.ActivationFunctionType
ALU = mybir.AluOpType


@dataclass
class Cfg:
    N: int = 50000
    C: int = 128
    PH: int = 64
    AH: int = 64
    DIM: int = 2
    M: int = 8            # cores
    T: int = 16           # 128-edge tiles per chunk
    TB: int = 4           # tiles per matmul block (block = 512 edges)
    SHIFT: float = 8.0
    EPS: float = 1e-12

    @property
    def NLOC(self):
        return self.N // self.M

    @property
    def CHUNK_E(self):
        return self.T * 128

    @property
    def OUT_ROWS(self):
        return self.NLOC + 1  # +1 trash row for padded scatter lanes


CFG = Cfg()


# ---------------------------------------------------------------- host pack
def _pack(x, pos, edge_index, cfg):
    """Sort/shard/chunk edges; returns per-core input dicts (minus weights)."""
    src = np.asarray(edge_index[0], np.int64)
    dst = np.asarray(edge_index[1], np.int64)
    order = np.argsort(dst, kind="stable")
    s_s = src[order]
    d_s = dst[order]
    posd = (pos[d_s] - pos[s_s]).astype(np.float16)  # [E, 2]

    NLOC = cfg.NLOC
    bounds = np.searchsorted(d_s, np.arange(cfg.M + 1) * NLOC)

    cores = []
    for c in range(cfg.M):
        lo, hi = bounds[c], bounds[c + 1]
        dloc = d_s[lo:hi] - c * NLOC
        deg = np.bincount(dloc, minlength=NLOC)
        nodes = np.nonzero(deg)[0]
        chunks = []  # (node_list, e0, e1) ; e relative to lo
        cur, cur_e, estart = [], 0, 0
        for n in nodes:
            dn = int(deg[n])
            assert dn <= cfg.CHUNK_E, f"degree {dn} exceeds chunk capacity"
            if len(cur) == 128 or cur_e + dn > cfg.CHUNK_E:
                chunks.append((cur, estart, estart + cur_e))
                estart += cur_e
                cur, cur_e = [], 0
            cur.append(int(n))
            cur_e += dn
        if cur:
            chunks.append((cur, estart, estart + cur_e))
        cores.append((lo, chunks, dloc))

    NCHUNK = max(len(ch) for _, ch, _ in cores) if cores else 1
    NCHUNK = max(NCHUNK, 1)

    in_maps = []
    for c in range(cfg.M):
        lo, chunks, dloc = cores[c]
        srcid = np.zeros((NCHUNK, 128, cfg.T), np.int32)
        dstid = np.zeros((NCHUNK, 128, cfg.T), np.int32)      # LOCAL dst ids
        dstloc = np.full((NCHUNK, 128, cfg.T), -1.0, np.float32)
        posdT = np.zeros((NCHUNK, cfg.DIM, cfg.CHUNK_E), np.float16)
        outrow = np.full((NCHUNK, 128), cfg.NLOC, np.int32)  # trash row
        for k, (nl, e0, e1) in enumerate(chunks):
            cnt = e1 - e0
            g0, g1 = lo + e0, lo + e1
            nla = np.asarray(nl, np.int64)
            loc = np.searchsorted(nla, dloc[e0:e1]).astype(np.float32)
            j = np.arange(cnt)
            t_idx = j >> 7
            lane = j & 127
            srcid[k, lane, t_idx] = s_s[g0:g1].astype(np.int32)
            dstid[k, lane, t_idx] = (d_s[g0:g1] - c * NLOC).astype(np.int32)
            dstloc[k, lane, t_idx] = loc
            posdT[k, :, :cnt] = posd[g0:g1].T
            outrow[k, : len(nl)] = nla.astype(np.int32)
        in_maps.append(
            dict(srcid=srcid, dstid=dstid, dstloc=dstloc, posdT=posdT,
                 outrow=outrow)
        )
    return in_maps, NCHUNK


# ---------------------------------------------------------------- program
def _build(cfg, nchunk):
    nc = bacc.Bacc(None, target_bir_lowering=False, num_devices=cfg.M)
    N, C, PH, AH, DIM = cfg.N, cfg.C, cfg.PH, cfg.AH, cfg.DIM
    NLOC = cfg.NLOC

    xT_d = nc.declare_dram_parameter("xT", [C, NLOC], f16, isOutput=False)
    wnode_d = nc.declare_dram_parameter("Wnode", [C, 2 * AH + C], f16, isOutput=False)
    wp1_d = nc.declare_dram_parameter("Wp1", [DIM, PH], f16, isOutput=False)
    wp2_d = nc.declare_dram_parameter("Wp2", [PH, C], f16, isOutput=False)
    wa1_d = nc.declare_dram_parameter("Wa1p", [C, AH], f16, isOutput=False)
    wa2_d = nc.declare_dram_parameter("Wa2", [AH, C], f16, isOutput=False)
    bias_d = nc.declare_dram_parameter("bias", [128, 5], f32, isOutput=False)
    src_d = nc.declare_dram_parameter("srcid", [nchunk, 128, cfg.T], i32, isOutput=False)
    dst_d = nc.declare_dram_parameter("dstid", [nchunk, 128, cfg.T], i32, isOutput=False)
    dl_d = nc.declare_dram_parameter("dstloc", [nchunk, 128, cfg.T], f32, isOutput=False)
    pd_d = nc.declare_dram_parameter("posdT", [nchunk, DIM, cfg.CHUNK_E], f16, isOutput=False)
    or_d = nc.declare_dram_parameter("outrow", [nchunk, 128], i32, isOutput=False)
    y_d = nc.declare_dram_parameter("y", [cfg.OUT_ROWS, C], f16, isOutput=True)

    U_loc = nc.dram_tensor("U_loc", [NLOC, AH], f16)          # x_c @ (W_dst@Wa1)
    vh_send = nc.dram_tensor("vh_send", [NLOC, AH + C], f16)  # x_c @ [W_src@Wa1 | W_lin]
    vh_full = nc.dram_tensor("vh_full", [N, AH + C], f16, addr_space="Shared")

    NB = cfg.T // cfg.TB  # blocks per chunk
    BLK = cfg.TB * 128

    with tile.TileContext(nc) as tc:
        with tc.tile_pool(name="const", bufs=1) as cp:
            wnode_s = cp.tile([C, 2 * AH + C], f16)
            nc.sync.dma_start(out=wnode_s[:], in_=wnode_d[:, :])
            wp1_s = cp.tile([DIM, PH], f16)
            nc.sync.dma_start(out=wp1_s[:], in_=wp1_d[:, :])
            wp2_s = cp.tile([PH, C], f16)
            nc.sync.dma_start(out=wp2_s[:], in_=wp2_d[:, :])
            wa1_s = cp.tile([C, AH], f16)
            nc.sync.dma_start(out=wa1_s[:], in_=wa1_d[:, :])
            wa2_s = cp.tile([AH, C], f16)
            nc.sync.dma_start(out=wa2_s[:], in_=wa2_d[:, :])
            bias_s = cp.tile([128, 5], f32)
            nc.sync.dma_start(out=bias_s[:], in_=bias_d[:, :])
            ident_s = cp.tile([128, 128], f16)
            make_identity(nc, ident_s[:])
            ident32_s = cp.tile([128, 128], f32)
            make_identity(nc, ident32_s[:])
            iota_i = cp.tile([128, 128], i32)
            nc.gpsimd.iota(iota_i[:], pattern=[[1, 128]], base=0, channel_multiplier=0)
            iota_s = cp.tile([128, 128], f16)
            nc.vector.tensor_copy(iota_s[:], iota_i[:])
            xT_s = cp.tile([C, NLOC], f16)
            nc.sync.dma_start(out=xT_s[:], in_=xT_d[:, :])

            # ---------------- phase 1: local node features U / VH ----------
            with tc.tile_pool(name="p1", bufs=3) as p1, \
                 tc.tile_pool(name="p1ps", bufs=2, space="PSUM") as p1ps:
                nt = ceil(NLOC / 128)
                for t in range(nt):
                    r0 = t * 128
                    rows = min(128, NLOC - r0)
                    uvh_p = p1ps.tile([128, 2 * AH + C], f32, tag="uvh")
                    nc.tensor.matmul(uvh_p[:rows, :], lhsT=xT_s[:, r0:r0 + rows],
                                     rhs=wnode_s[:], start=True, stop=True)
                    uvh_s = p1.tile([128, 2 * AH + C], f16, tag="uvhs")
                    nc.scalar.activation(uvh_s[:rows, :], uvh_p[:rows, :], AF.Copy)
                    nc.sync.dma_start(out=U_loc[r0:r0 + rows, :], in_=uvh_s[:rows, 0:AH])
                    nc.sync.dma_start(out=vh_send[r0:r0 + rows, :], in_=uvh_s[:rows, AH:])

            # ---------------- all-gather VH across cores ----------
            nc.gpsimd.collective_compute(
                "AllGather",
                mybir.AluOpType.bypass,
                replica_groups=[list(range(cfg.M))],
                ins=[vh_send[:, :]],
                outs=[vh_full[:, :]],
            )

            # ---------------- phase 2: edges ----------------
            with tc.tile_pool(name="eb", bufs=3) as eb, \
                 tc.tile_pool(name="ebg", bufs=3) as ebg, \
                 tc.tile_pool(name="ps_acc", bufs=2, space="PSUM") as ps_acc, \
                 tc.tile_pool(name="ps_b", bufs=1, space="PSUM") as ps_b, \
                 tc.tile_pool(name="ps_c", bufs=1, space="PSUM") as ps_c, \
                 tc.tile_pool(name="ps_m", bufs=1, space="PSUM") as ps_m, \
                 tc.tile_pool(name="ps_n", bufs=1, space="PSUM") as ps_n, \
                 tc.tile_pool(name="ps_t", bufs=2, space="PSUM") as ps_t:
                for k in range(nchunk):
                    src_s = eb.tile([128, cfg.T], i32, tag="src")
                    nc.sync.dma_start(out=src_s[:], in_=src_d[k, :, :])
                    dst_s = eb.tile([128, cfg.T], i32, tag="dst")
                    nc.sync.dma_start(out=dst_s[:], in_=dst_d[k, :, :])
                    dl_s = eb.tile([128, cfg.T], f32, tag="dl")
                    nc.sync.dma_start(out=dl_s[:], in_=dl_d[k, :, :])
                    pd_s = eb.tile([DIM, cfg.CHUNK_E], f16, tag="pd")
                    nc.sync.dma_start(out=pd_s[:], in_=pd_d[k, :, :])
                    or_s = eb.tile([128, 1], i32, tag="or")
                    nc.sync.dma_start(out=or_s[:], in_=or_d[k, :, None])

                    acc_p = ps_acc.tile([128, 2 * C], f32, tag="acc")

                    for b in range(NB):
                        esl = slice(b * BLK, (b + 1) * BLK)
                        # gathers for this block, one [128,1]-offset DMA per tile
                        vhgs, ugs = [], []
                        for tt in range(cfg.TB):
                            ti = b * cfg.TB + tt
                            vhg_t = ebg.tile([128, AH + C], f16, tag=f"vhg{tt}")
                            nc.gpsimd.indirect_dma_start(
                                out=vhg_t[:], out_offset=None, in_=vh_full[:],
                                in_offset=IndirectOffsetOnAxis(
                                    ap=src_s[:, ti:ti + 1], axis=0))
                            vhgs.append(vhg_t)
                            ug_t = ebg.tile([128, AH], f16, tag=f"ug{tt}")
                            nc.gpsimd.indirect_dma_start(
                                out=ug_t[:], out_offset=None, in_=U_loc[:],
                                in_offset=IndirectOffsetOnAxis(
                                    ap=dst_s[:, ti:ti + 1], axis=0))
                            ugs.append(ug_t)

                        # pos MLP
                        tp1_p = ps_m.tile([PH, BLK], f32, tag="tp1")
                        nc.tensor.matmul(tp1_p[:], lhsT=wp1_s[:],
                                         rhs=pd_s[:, esl], start=True, stop=True)
                        tp1_s = eb.tile([PH, BLK], f16, tag="tp1s")
                        nc.scalar.activation(tp1_s[:], tp1_p[:], AF.Relu, bias=bias_s[0:PH, 0:1])
                        del_p = ps_b.tile([C, BLK], f32, tag="delp")
                        nc.tensor.matmul(del_p[:], lhsT=wp2_s[:],
                                         rhs=tp1_s[:], start=True, stop=True)
                        del_s = eb.tile([C, BLK], f16, tag="dels")
                        nc.scalar.activation(del_s[:], del_p[:], AF.Relu, bias=bias_s[:, 1:2])

                        # attn layer 1: z1 = Wa1^T delta + (U[dst]-V[src])^T.
                        # The per-tile gd transposes accumulate straight into
                        # the z1 PSUM group (PE executes in program order, so
                        # the start=True matmul lands first).
                        z1_p = ps_n.tile([AH, BLK], f32, tag="z1")
                        nc.tensor.matmul(z1_p[:], lhsT=wa1_s[:],
                                         rhs=del_s[:], start=True, stop=False)
                        for tt in range(cfg.TB):
                            gd_s = eb.tile([128, AH], f32, tag="gd")
                            nc.vector.tensor_tensor(gd_s[:], ugs[tt][:], vhgs[tt][:, 0:AH],
                                                    op=ALU.subtract)
                            csl = slice(tt * 128, (tt + 1) * 128)
                            nc.tensor.matmul(z1_p[:, csl], lhsT=gd_s[:], rhs=ident32_s[:],
                                             is_transpose=True, start=False, stop=True,
                                             skip_group_check=True)
                        ta_s = eb.tile([AH, BLK], f16, tag="ta")
                        nc.scalar.activation(ta_s[:], z1_p[:], AF.Relu, bias=bias_s[0:AH, 2:3])

                        # attn layer 2 + exp
                        al_p = ps_c.tile([C, BLK], f32, tag="al")
                        nc.tensor.matmul(al_p[:], lhsT=wa2_s[:],
                                         rhs=ta_s[:], start=True, stop=True)
                        ar_s = eb.tile([C, BLK], f32, tag="ar")
                        nc.scalar.activation(ar_s[:], al_p[:], AF.Relu, bias=bias_s[:, 3:4])
                        e_s = eb.tile([C, BLK], f16, tag="e")
                        nc.scalar.activation(e_s[:], ar_s[:], AF.Exp, bias=bias_s[:, 4:5])
                        ew2_s = eb.tile([C, BLK], f16, tag="ew2")
                        nc.vector.tensor_tensor(ew2_s[:], e_s[:], del_s[:], op=ALU.mult)

                        # per-tile: transpose, assemble [ew | e]^T, indicator, seg-matmul
                        for tt in range(cfg.TB):
                            ti = b * cfg.TB + tt
                            csl = slice(tt * 128, (tt + 1) * 128)
                            eT_p = ps_t.tile([128, 128], f16, tag="tr")
                            nc.tensor.transpose(eT_p[:], e_s[:, csl], ident_s[:])
                            ew2T_p = ps_t.tile([128, 128], f16, tag="tr")
                            nc.tensor.transpose(ew2T_p[:], ew2_s[:, csl], ident_s[:])
                            ewe_s = eb.tile([128, 2 * C], f16, tag="ewe")
                            nc.vector.tensor_copy(ewe_s[:, C:], eT_p[:])
                            tmp_s = eb.tile([128, C], f16, tag="tmp")
                            nc.vector.tensor_tensor(tmp_s[:], eT_p[:], vhgs[tt][:, AH:],
                                                    op=ALU.mult)
                            nc.vector.tensor_tensor(ewe_s[:, 0:C], tmp_s[:], ew2T_p[:],
                                                    op=ALU.add)
                            ind_s = eb.tile([128, 128], f16, tag="ind")
                            nc.vector.tensor_scalar(ind_s[:], iota_s[:], dl_s[:, ti:ti + 1],
                                                    None, op0=ALU.is_equal)
                            nc.tensor.matmul(acc_p[:], lhsT=ind_s[:],
                                             rhs=ewe_s[:],
                                             start=(ti == 0), stop=(ti == cfg.T - 1))

                    # finalize chunk
                    sp_s = eb.tile([128, C], f32, tag="sp")
                    nc.vector.tensor_scalar_add(sp_s[:], acc_p[:, C:], cfg.EPS)
                    rp_s = eb.tile([128, C], f32, tag="rp")
                    nc.vector.reciprocal(rp_s[:], sp_s[:])
                    o_s = eb.tile([128, C], f32, tag="o")
                    nc.vector.tensor_tensor(o_s[:], acc_p[:, 0:C], rp_s[:], op=ALU.mult)
                    o2_s = eb.tile([128, C], f16, tag="o2")
                    nc.scalar.activation(o2_s[:], o_s[:], AF.Relu)
                    nc.gpsimd.indirect_dma_start(
                        out=y_d[:], out_offset=IndirectOffsetOnAxis(ap=or_s[:, :1], axis=0),
                        in_=o2_s[:], in_offset=None)
    nc.finalize()
    return nc


def _build_inputs(inputs, cfg):
    x = np.asarray(inputs["x"], np.float32)
    pos = np.ascontiguousarray(np.asarray(inputs["pos"], np.float32))
    W_lin = np.asarray(inputs["W_lin"], np.float32)
    W_src = np.asarray(inputs["W_src"], np.float32)
    W_dst = np.asarray(inputs["W_dst"], np.float32)
    Wp1 = np.asarray(inputs["Wp1"], np.float32)
    bp1 = np.asarray(inputs["bp1"], np.float32)
    Wp2 = np.asarray(inputs["Wp2"], np.float32)
    bp2 = np.asarray(inputs["bp2"], np.float32)
    Wa1 = np.asarray(inputs["Wa1"], np.float32)
    ba1 = np.asarray(inputs["ba1"], np.float32)
    Wa2 = np.asarray(inputs["Wa2"], np.float32)
    ba2 = np.asarray(inputs["ba2"], np.float32)

    Wda = (W_dst @ Wa1).astype(np.float16)   # [C, AH]
    Wsa = (W_src @ Wa1).astype(np.float16)
    wnode = np.concatenate([Wda, Wsa, W_lin.astype(np.float16)], axis=1)
    bias = np.zeros((128, 5), np.float32)
    bias[: cfg.PH, 0] = bp1
    bias[: cfg.C, 1] = bp2
    bias[: cfg.AH, 2] = ba1
    bias[: cfg.C, 3] = ba2
    bias[:, 4] = -cfg.SHIFT

    packs, nchunk = _pack(x, pos, inputs["edge_index"], cfg)
    common = dict(Wnode=np.ascontiguousarray(wnode),
                  Wp1=np.ascontiguousarray(Wp1.astype(np.float16)),
                  Wp2=np.ascontiguousarray(Wp2.astype(np.float16)),
                  Wa1p=np.ascontiguousarray(Wa1.astype(np.float16)),
                  Wa2=np.ascontiguousarray(Wa2.astype(np.float16)),
                  bias=bias)
    xh = x.astype(np.float16)
    in_maps = []
    for c, p in enumerate(packs):
        xT_c = np.ascontiguousarray(xh[c * cfg.NLOC:(c + 1) * cfg.NLOC, :].T)
        in_maps.append(dict(common, xT=xT_c, **p))
    return in_maps, nchunk


def kernel(**inputs):
    cfg = CFG
    in_maps, nchunk = _build_inputs(inputs, cfg)
    nc = _build(cfg, nchunk)
    res = run_bass_kernel_spmd(nc, in_maps, list(range(cfg.M)))
    y = np.concatenate([res.results[c]["y"][: cfg.NLOC] for c in range(cfg.M)], axis=0)
    return y.astype(np.float32)
